# revision 3
# baseline (speedup 1.0000x reference)
"""Trainium2 Bass kernel for BiDenseAdjacency: y[b] = feats[b] @ w @ feats[b]^T + bias.

Full inputs in, full outputs out. Data-parallel over batch: B=32 batches split
4-per-core across 8 NeuronCores; w/b replicated. Per core and batch:
  T   = feats[b]^T                (PE transposes, [F=128, N=1024] in SBUF)
  vwT = w^T-contract:  vwT[g,n] = sum_f w[f,g] T[f,n]      (matmul, lhsT=w)
  y   = vwT^T-contract: y[n,m] = sum_g vwT[g,n] T[g,m] + b (matmul, lhsT=vwT cols)
Output DMA (16 MiB/core) is the roofline; matmuls run as float32r to keep the
tensor engine off the critical path.
"""

import numpy as np

import concourse.mybir as mybir
import concourse.tile as tile
from concourse import bacc
from concourse.bass_utils import run_bass_kernel_spmd
from concourse.masks import make_identity

B, N, F = 32, 1024, 128
N_CORES = 8
BPC = B // N_CORES  # batches per core
P = 128             # partitions
NT = N // P         # row tiles per batch
H = 512             # matmul moving-dim chunk (PSUM bank limit for 4-byte)

F32 = mybir.dt.float32
MM_DT = mybir.dt.float32r  # matmul compute dtype (1 cyc/row vs 4 for float32)


def build_nc(mm_dt=MM_DT):
    nc = bacc.Bacc("TRN2", target_bir_lowering=False, debug=False,
                   num_devices=N_CORES)
    feats_d = nc.dram_tensor("feats", [BPC, N, F], F32, kind="ExternalInput")
    w_d = nc.dram_tensor("w", [F, F], F32, kind="ExternalInput")
    b_d = nc.dram_tensor("b", [1], F32, kind="ExternalInput")
    y_d = nc.dram_tensor("y", [BPC, N, N], F32, kind="ExternalOutput")

    with tile.TileContext(nc) as tc:
        with (
            tc.tile_pool(name="const", bufs=1) as cpool,
            tc.tile_pool(name="fload", bufs=2) as fpool,
            tc.tile_pool(name="tbuf", bufs=2) as tpool,
            tc.tile_pool(name="vwbuf", bufs=2) as vpool,
            tc.tile_pool(name="ybuf", bufs=2) as ypool,
            tc.tile_pool(name="ps_t", bufs=1, space="PSUM") as ps_t,
            tc.tile_pool(name="ps_v", bufs=1, space="PSUM") as ps_v,
            tc.tile_pool(name="ps_y", bufs=2, space="PSUM") as ps_y,
        ):
            w_s = cpool.tile([F, F], F32)
            nc.sync.dma_start(w_s[:], w_d[:])
            # matmul operands must be written in the matmul dtype (fp32r
            # rounding happens at the producing instruction per the BIR
            # verifier), so keep fp32r copies of w / T / vwT in SBUF.
            w_r = cpool.tile([F, F], mm_dt, tag="w_r")
            nc.vector.tensor_copy(w_r[:], w_s[:])
            bias_s = cpool.tile([P, 1], F32)
            nc.sync.dma_start(bias_s[:], b_d[:].to_broadcast((P, 1)))
            ident = cpool.tile([P, P], F32)
            make_identity(nc, ident[:])

            for bi in range(BPC):
                # feats[bi] -> SBUF, p-major: f_t[p, i, f] = feats[bi, i*P+p, f]
                f_t = fpool.tile([P, NT, F], F32)
                nc.sync.dma_start(
                    f_t[:], feats_d[bi].rearrange("(i p) f -> p i f", p=P)
                )

                # T = feats[bi]^T via 8 PE transposes into one PSUM tile
                t_ps = ps_t.tile([P, N], F32)
                for i in range(NT):
                    nc.tensor.transpose(
                        t_ps[:, i * P:(i + 1) * P], f_t[:, i, :], ident[:]
                    )
                t_s = tpool.tile([P, N], mm_dt)
                nc.vector.tensor_copy(t_s[:], t_ps[:])

                # vwT[g, n] = sum_f w[f, g] * T[f, n]
                vw_ps = ps_v.tile([P, N], F32)
                for h in range(N // H):
                    nc.tensor.matmul(
                        vw_ps[:, h * H:(h + 1) * H],
                        w_r[:],
                        t_s[:, h * H:(h + 1) * H],
                    )
                vw_s = vpool.tile([P, N], mm_dt)
                nc.scalar.activation(
                    vw_s[:], vw_ps[:], mybir.ActivationFunctionType.Copy
                )

                # y[n, m] = sum_g vwT[g, n] * T[g, m] + bias
                y_s = ypool.tile([P, NT, N], F32)
                for i in range(NT):
                    y_ps = ps_y.tile([P, N], F32)
                    for h in range(N // H):
                        nc.tensor.matmul(
                            y_ps[:, h * H:(h + 1) * H],
                            vw_s[:, i * P:(i + 1) * P],
                            t_s[:, h * H:(h + 1) * H],
                        )
                    if i % 2 == 0:
                        nc.scalar.activation(
                            y_s[:, i, :], y_ps[:],
                            mybir.ActivationFunctionType.Identity,
                            bias=bias_s[:],
                        )
                    else:
                        nc.vector.tensor_scalar_add(y_s[:, i, :], y_ps[:], bias_s[:])

                nc.sync.dma_start(
                    y_d[bi].rearrange("(i p) m -> p i m", p=P), y_s[:]
                )

    nc.compile()
    return nc


_NC_CACHE = {}


def _get_nc(mm_dt=MM_DT):
    key = str(mm_dt)
    if key not in _NC_CACHE:
        _NC_CACHE[key] = build_nc(mm_dt)
    return _NC_CACHE[key]


def run_on_cores(feats, w, b, mm_dt=MM_DT, trace=False):
    nc = _get_nc(mm_dt)
    feats = np.ascontiguousarray(feats, dtype=np.float32)
    w = np.ascontiguousarray(w, dtype=np.float32)
    b = np.ascontiguousarray(b, dtype=np.float32)
    in_maps = [
        {"feats": feats[c * BPC:(c + 1) * BPC], "w": w, "b": b}
        for c in range(N_CORES)
    ]
    res = run_bass_kernel_spmd(nc, in_maps, core_ids=list(range(N_CORES)),
                               trace=trace)
    y = np.concatenate([res.results[c]["y"] for c in range(N_CORES)], axis=0)
    return y, res


def kernel(adjMs, feats, w, b):
    y, _ = run_on_cores(feats, w, b)
    feats = np.asarray(feats, dtype=np.float32)
    return y, feats


# revision 4
# speedup vs baseline: 1.1081x; 1.1081x over previous
"""Trainium2 Bass kernel for BiDenseAdjacency: y[b] = feats[b] @ w @ feats[b]^T + bias.

Full inputs in, full outputs out. Data-parallel over batch: B=32 batches split
4-per-core across 8 NeuronCores; w/b replicated. Per core and batch:
  T   = feats[b]^T                (PE transposes, [F=128, N=1024] in SBUF)
  vwT = w^T-contract:  vwT[g,n] = sum_f w[f,g] T[f,n]      (matmul, lhsT=w)
  y   = vwT^T-contract: y[n,m] = sum_g vwT[g,n] T[g,m] + b (matmul, lhsT=vwT cols)
Output DMA (16 MiB/core) is the roofline; matmuls run as float32r to keep the
tensor engine off the critical path. Output stores go out in half-batch (2 MiB)
chunks alternating between the SP and ACT HWDGE rings so the store stream
starts early and drains with minimal exposed tail.
"""

import numpy as np

import concourse.mybir as mybir
import concourse.tile as tile
from concourse import bacc
from concourse.bass_utils import run_bass_kernel_spmd
from concourse.masks import make_identity

B, N, F = 32, 1024, 128
N_CORES = 8
BPC = B // N_CORES  # batches per core
P = 128             # partitions
NT = N // P         # row tiles per batch
H = 512             # matmul moving-dim chunk (PSUM bank limit for 4-byte)

F32 = mybir.dt.float32
MM_DT = mybir.dt.float32r  # matmul compute dtype (1 cyc/row vs 4 for float32)


def build_nc(mm_dt=MM_DT):
    nc = bacc.Bacc("TRN2", target_bir_lowering=False, debug=False,
                   num_devices=N_CORES)
    feats_d = nc.dram_tensor("feats", [BPC, N, F], F32, kind="ExternalInput")
    w_d = nc.dram_tensor("w", [F, F], F32, kind="ExternalInput")
    b_d = nc.dram_tensor("b", [1], F32, kind="ExternalInput")
    y_d = nc.dram_tensor("y", [BPC, N, N], F32, kind="ExternalOutput")

    with tile.TileContext(nc) as tc:
        with (
            tc.tile_pool(name="const", bufs=1) as cpool,
            tc.tile_pool(name="fload", bufs=BPC) as fpool,
            tc.tile_pool(name="tbuf", bufs=2) as tpool,
            tc.tile_pool(name="vwbuf", bufs=2) as vpool,
            tc.tile_pool(name="ybuf", bufs=4) as ypool,
            tc.tile_pool(name="ps_t", bufs=1, space="PSUM") as ps_t,
            tc.tile_pool(name="ps_v", bufs=1, space="PSUM") as ps_v,
            tc.tile_pool(name="ps_y", bufs=2, space="PSUM") as ps_y,
        ):
            # feats loads first so the row-tile pipeline starts ASAP.
            # Contiguous layout: f_t[p, j, f] = feats[bi, p*NT + j, f]
            # (4 KiB contiguous per partition -> full-rate DMA).
            f_tiles = []
            for bi in range(BPC):
                f_t = fpool.tile([P, NT, F], F32, tag="f_t")
                nc.sync.dma_start(
                    f_t[:], feats_d[bi].rearrange("(p j) f -> p j f", j=NT)
                )
                f_tiles.append(f_t)

            w_s = cpool.tile([F, F], F32)
            nc.scalar.dma_start(w_s[:], w_d[:])
            # matmul operands must be written in the matmul dtype (fp32r
            # rounding happens at the producing instruction per the BIR
            # verifier), so keep fp32r copies of w / T / vwT in SBUF.
            w_r = cpool.tile([F, F], mm_dt, tag="w_r")
            nc.vector.tensor_copy(w_r[:], w_s[:])
            bias_s = cpool.tile([P, 1], F32)
            nc.scalar.dma_start(bias_s[:], b_d[:].to_broadcast((P, 1)))
            ident = cpool.tile([P, P], F32)
            make_identity(nc, ident[:])

            for bi in range(BPC):
                f_t = f_tiles[bi]

                # T = feats[bi]^T. Transpose j gives columns {p*NT + j};
                # one scatter-copy reassembles natural column order.
                t_ps = ps_t.tile([P, NT, P], F32)
                for j in range(NT):
                    nc.tensor.transpose(t_ps[:, j, :], f_t[:, j, :], ident[:])
                t_s = tpool.tile([P, N], mm_dt)
                nc.vector.tensor_copy(
                    t_s[:].rearrange("g (p j) -> g j p", j=NT), t_ps[:]
                )

                # vwT[g, n] = sum_f w[f, g] * T[f, n]
                vw_ps = ps_v.tile([P, N], F32)
                for h in range(N // H):
                    nc.tensor.matmul(
                        vw_ps[:, h * H:(h + 1) * H],
                        w_r[:],
                        t_s[:, h * H:(h + 1) * H],
                    )
                vw_s = vpool.tile([P, N], mm_dt)
                nc.scalar.activation(
                    vw_s[:], vw_ps[:], mybir.ActivationFunctionType.Copy
                )

                # y[n, m] = sum_g vwT[g, n] * T[g, m] + bias
                # Row tiles i here are natural row blocks: rows i*P..i*P+P-1
                # of y[bi] come from lhsT = vwT columns i*P..i*P+P-1.
                half_tiles = NT // 2
                for half in range(2):
                    y_s = ypool.tile([P, half_tiles, N], F32, tag="y_s")
                    for k in range(half_tiles):
                        i = half * half_tiles + k
                        y_ps = ps_y.tile([P, N], F32)
                        for h in range(N // H):
                            nc.tensor.matmul(
                                y_ps[:, h * H:(h + 1) * H],
                                vw_s[:, i * P:(i + 1) * P],
                                t_s[:, h * H:(h + 1) * H],
                            )
                        if i % 2 == 0:
                            nc.scalar.activation(
                                y_s[:, k, :], y_ps[:],
                                mybir.ActivationFunctionType.Identity,
                                bias=bias_s[:],
                            )
                        else:
                            nc.vector.tensor_scalar_add(
                                y_s[:, k, :], y_ps[:], bias_s[:]
                            )

                    dram_half = y_d[bi].rearrange("(i p) m -> p i m", p=P)[
                        :, half * half_tiles:(half + 1) * half_tiles, :
                    ]
                    if bi < BPC - 1:
                        eng = nc.sync if half == 0 else nc.scalar
                        eng.dma_start(dram_half, y_s[:])
                    else:
                        # Last batch: quarter-chunks so the tail drains fast.
                        q = half_tiles // 2
                        for qi in range(2):
                            eng = nc.sync if (half + qi) % 2 == 0 else nc.scalar
                            eng.dma_start(
                                dram_half[:, qi * q:(qi + 1) * q, :],
                                y_s[:, qi * q:(qi + 1) * q, :],
                            )

    nc.compile()
    return nc


_NC_CACHE = {}


def _get_nc(mm_dt=MM_DT):
    key = str(mm_dt)
    if key not in _NC_CACHE:
        _NC_CACHE[key] = build_nc(mm_dt)
    return _NC_CACHE[key]


def run_on_cores(feats, w, b, mm_dt=MM_DT, trace=False):
    nc = _get_nc(mm_dt)
    feats = np.ascontiguousarray(feats, dtype=np.float32)
    w = np.ascontiguousarray(w, dtype=np.float32)
    b = np.ascontiguousarray(b, dtype=np.float32)
    in_maps = [
        {"feats": feats[c * BPC:(c + 1) * BPC], "w": w, "b": b}
        for c in range(N_CORES)
    ]
    res = run_bass_kernel_spmd(nc, in_maps, core_ids=list(range(N_CORES)),
                               trace=trace)
    y = np.concatenate([res.results[c]["y"] for c in range(N_CORES)], axis=0)
    return y, res


def kernel(adjMs, feats, w, b):
    y, _ = run_on_cores(feats, w, b)
    feats = np.asarray(feats, dtype=np.float32)
    return y, feats
